# revision 9
# baseline (speedup 1.0000x reference)
"""NeRank (BiLSTM network-embedding + CNN ranking) Trainium2 kernel.

Strategy (8-core SPMD, full inputs in / full output out):
  - Host gathers embedding rows and LSTM input sequences (per sharding hint).
  - The two BiLSTMs (u-weights, v-weights) run on device in a transposed
    layout: state h/c stored as [128 part(hidden-chunk), 2, batch-cols]; per
    step the gate pre-activations are built in PSUM by fp32r matmuls
    (Wih.T @ x_t accumulated with Whh.T @ h), then ACT applies
    sigmoid/tanh with the per-gate bias fused into the activation, and DVE
    does the cell updates with exact per-core length masking
    (copy_predicated), so ragged sequences are handled by processing only
    a sorted active prefix each step.
  - Sequences are globally sorted by length and dealt round-robin across
    cores so all cores share one SPMD program with per-step active widths
    baked in at build time (kernel is specialized + compiled per call,
    cached on the derived schedule).
  - v-LSTM runs over the deduplicated set of queries referenced by
    qpos_v/qpos_neg; consumers gather from the returned table on the host.
  - The ranking convnet (OC=32 einsum convs) runs on device from the
    final hidden states; the tiny scalar reductions (ne_loss dots,
    log-sigmoids) are finished on host in float64.
"""

import os
import numpy as np

NCORES = 8
D = 256
H = 256
KC = 2  # hidden/input 128-chunks

_PROG_CACHE = {}
LAST_EXEC_NS = None


# ---------------------------------------------------------------- host utils

def _logsig(x):
    x = np.asarray(x, dtype=np.float64)
    return np.where(x > 0, -np.log1p(np.exp(-np.abs(x))), x - np.log1p(np.exp(-np.abs(x))))


def _deal(qidx, lens, L):
    """Sort by length desc, deal round-robin over cores.

    Returns (percore_qidx [8,n/8], percore_lens [8,n/8],
             placement (core,col) per original position,
             a_ceil [L] shared active width, counts [8,L] per-core counts).
    """
    n = len(qidx)
    assert n % NCORES == 0
    order = np.argsort(-lens, kind="stable")
    pos = np.empty(n, dtype=np.int64)
    pos[order] = np.arange(n)
    cores = pos % NCORES
    cols = pos // NCORES
    ncol = n // NCORES
    pq = np.zeros((NCORES, ncol), dtype=np.int64)
    pl = np.zeros((NCORES, ncol), dtype=np.int64)
    pq[cores, cols] = qidx
    pl[cores, cols] = lens
    ts = np.arange(L)
    g = (lens[None, :] > ts[:, None]).sum(axis=1)  # [L]
    a_ceil = np.minimum(-(-g // NCORES), ncol).astype(np.int64)
    counts = (pl[:, None, :] > ts[None, :, None]).sum(axis=2)  # [8, L]
    return pq, pl, (cores, cols), a_ceil, counts


def _pack_wT(W):
    # W: [4H, D] -> lhsT layout [128, 2, 4H]: [p, kc, gd] = W[gd, kc*128+p]
    WT = np.ascontiguousarray(W.T)  # [D, 4H]
    return np.ascontiguousarray(WT.reshape(KC, 128, W.shape[0]).transpose(1, 0, 2))


def _pack_rows_T(Erows):
    # Erows: [n, 256] -> [128, 2, n]
    return np.ascontiguousarray(Erows.T.reshape(KC, 128, Erows.shape[0]).transpose(1, 0, 2))


def _unpack_hT(h):
    # h: [128, 2, n] -> [n, 256]
    return np.ascontiguousarray(h.transpose(2, 1, 0).reshape(h.shape[2], 256))


# ---------------------------------------------------------------- program

def _build_program(L, Nv, n1, n2, sched, nbufs):
    import concourse.bacc as bacc
    import concourse.mybir as mybir
    import concourse.tile as tile

    F32 = mybir.dt.float32
    F32R = mybir.dt.float32r
    AF = mybir.ActivationFunctionType
    ALU = mybir.AluOpType

    NU = n1 + n2
    NB = NU + Nv

    nc = bacc.Bacc("TRN2", target_bir_lowering=False, debug=False,
                   num_devices=NCORES)

    x_in = nc.dram_tensor("x_in", [L, 128, KC, NB], F32R, kind="ExternalInput")
    wname = ["wih_uf", "whh_uf", "wih_vf", "whh_vf",
             "wih_ub", "whh_ub", "wih_vb", "whh_vb"]
    wdr = {n: nc.dram_tensor(n, [128, KC, 4 * H], F32R, kind="ExternalInput")
           for n in wname}
    bdr = {n: nc.dram_tensor(n, [128, 8], F32, kind="ExternalInput")
           for n in ["b_uf", "b_vf", "b_ub", "b_vb"]}
    iota_in = nc.dram_tensor("iota", [128, KC, NB], F32, kind="ExternalInput")
    cnts_in = nc.dram_tensor("cnts", [128, L, 3], F32, kind="ExternalInput")
    er_in = nc.dram_tensor("er", [128, KC, n2], F32, kind="ExternalInput")
    ea_in = nc.dram_tensor("ea", [128, KC, n2], F32, kind="ExternalInput")
    eacc_in = nc.dram_tensor("eacc", [128, KC, n2], F32, kind="ExternalInput")
    k1_in = nc.dram_tensor("k1t", [128, KC, 32], F32, kind="ExternalInput")
    k2_in = nc.dram_tensor("k2t", [128, KC, 2, 32], F32, kind="ExternalInput")
    k3_in = nc.dram_tensor("k3t", [128, KC, 3, 32], F32, kind="ExternalInput")
    cb_in = nc.dram_tensor("cbt", [32, 3], F32, kind="ExternalInput")
    fw_in = nc.dram_tensor("fwt", [32, 3], F32, kind="ExternalInput")

    hu_out = nc.dram_tensor("hu_out", [128, KC, NU], F32, kind="ExternalOutput")
    hv_out = nc.dram_tensor("hv_out", [128, KC, Nv], F32, kind="ExternalOutput")
    sc_out = nc.dram_tensor("sc_out", [2, n2], F32, kind="ExternalOutput")

    fbias_low, fbias_high = sched["fbias"]

    with tile.TileContext(nc) as tc:
        with (
            tc.tile_pool(name="const", bufs=1) as cpool,
            tc.tile_pool(name="wts", bufs=1) as wpool,
            tc.tile_pool(name="xs", bufs=nbufs["x"]) as xpool,
            tc.tile_pool(name="gates", bufs=nbufs["g"]) as gpool,
            tc.tile_pool(name="ths", bufs=2) as thpool,
            tc.tile_pool(name="tmps", bufs=nbufs["tmp"]) as tpool,
            tc.tile_pool(name="masks", bufs=2) as mpool,
            tc.tile_pool(name="psu", bufs=nbufs["psu"], space="PSUM") as psupool,
            tc.tile_pool(name="psv", bufs=nbufs["psv"], space="PSUM") as psvpool,
            tc.tile_pool(name="convs", bufs=1) as convpool,
            tc.tile_pool(name="cps", bufs=1, space="PSUM") as cpspool,
        ):
            # ------- persistent state + constants
            iota_t = cpool.tile([128, KC, NB], F32)
            nc.sync.dma_start(iota_t[:], iota_in[:])
            cnts_t = cpool.tile([128, L, 3], F32)
            nc.sync.dma_start(cnts_t[:], cnts_in[:])
            bias_t = {}
            for n in bdr:
                bias_t[n] = cpool.tile([128, 8], F32, tag=f"bias_{n}", name=f"bias_{n}")
                nc.sync.dma_start(bias_t[n][:], bdr[n][:])

            hT = cpool.tile([128, KC, NB], F32, tag="hT")
            hTr = cpool.tile([128, KC, NB], F32R, tag="hTr")
            cT = cpool.tile([128, KC, NB], F32, tag="cT")
            hsum = cpool.tile([128, KC, NB], F32, tag="hsum")
            nc.gpsimd.memset(hT[:], 0.0)
            nc.gpsimd.memset(cT[:], 0.0)
            nc.vector.tensor_copy(hTr[:], cT[:])

            def load_weights(dirn):
                w = {}
                for lstm in ("u", "v"):
                    for kind in ("wih", "whh"):
                        t = wpool.tile([128, KC, 4 * H], F32R, tag=f"{kind}_{lstm}", name=f"{kind}_{lstm}{dirn}")
                        nc.sync.dma_start(t[:], wdr[f"{kind}_{lstm}{dirn}"][:])
                        w[f"{kind}_{lstm}"] = t
                return w

            GATE_FUNC = [AF.Sigmoid, AF.Sigmoid, AF.Tanh, AF.Sigmoid]  # i,f,g,o

            def lstm_step(w, bias_u, bias_v, t, a1, a2, av):
                u_on = (a1 + a2) > 0
                v_on = av > 0
                Wv = 0
                if v_on:
                    Wv = min(max(int(av), 256), Nv)
                    Wv += Wv % 2
                xlo = 0 if u_on else NU
                xhi = NU + Wv if v_on else NU

                xt = xpool.tile([128, KC, NB], F32R, tag="xt")
                for k in range(KC):
                    nc.sync.dma_start(xt[:, k, xlo:xhi], x_in[t, :, k, xlo:xhi])

                gts = [gpool.tile([128, KC, NB], F32, tag=f"g{i}", name=f"g{i}_{t}")
                       for i in range(4)]

                for g in range(4):
                    for kc in range(KC):
                        m = 2 * g + kc
                        ms = slice(m * 128, (m + 1) * 128)
                        if u_on:
                            pu = psupool.tile([128, NU], F32, tag="pu")
                            for k in range(KC):
                                nc.tensor.matmul(pu[:], w["wih_u"][:, k, ms],
                                                 xt[:, k, 0:NU],
                                                 start=(k == 0), stop=False)
                            for k in range(KC):
                                nc.tensor.matmul(pu[:], w["whh_u"][:, k, ms],
                                                 hTr[:, k, 0:NU],
                                                 start=False, stop=(k == KC - 1))
                            nc.scalar.activation(gts[g][:, kc, 0:NU], pu[:],
                                                 GATE_FUNC[g],
                                                 bias=bias_u[:, m:m + 1])
                        if v_on:
                            pv = psvpool.tile([128, Wv], F32, tag="pv")
                            for k in range(KC):
                                nc.tensor.matmul(pv[:], w["wih_v"][:, k, ms],
                                                 xt[:, k, NU:NU + Wv],
                                                 start=(k == 0), stop=False)
                            for k in range(KC):
                                nc.tensor.matmul(pv[:], w["whh_v"][:, k, ms],
                                                 hTr[:, k, NU:NU + Wv],
                                                 start=False, stop=(k == KC - 1))
                            nc.scalar.activation(gts[g][:, kc, NU:NU + Wv], pv[:],
                                                 GATE_FUNC[g],
                                                 bias=bias_v[:, m:m + 1])

                segs = []
                if a1 > 0:
                    segs.append((0, int(a1), 0))
                if a2 > 0:
                    segs.append((n1, int(a2), 1))
                if av > 0:
                    segs.append((NU, int(av), 2))

                mask = mpool.tile([128, KC, NB], mybir.dt.uint8, tag="mask")
                # c update
                for (o, a, si) in segs:
                    sl = slice(o, o + a)
                    nc.vector.tensor_scalar(mask[:, :, sl], iota_t[:, :, sl],
                                            cnts_t[:, t, si:si + 1], None,
                                            ALU.is_lt)
                    t1 = tpool.tile([128, KC, NB], F32, tag="tmp")
                    t2 = tpool.tile([128, KC, NB], F32, tag="tmp")
                    nc.vector.tensor_mul(t1[:, :, sl], gts[1][:, :, sl], cT[:, :, sl])
                    nc.vector.tensor_mul(t2[:, :, sl], gts[0][:, :, sl], gts[2][:, :, sl])
                    nc.vector.tensor_add(t1[:, :, sl], t1[:, :, sl], t2[:, :, sl])
                    nc.vector.copy_predicated(cT[:, :, sl], mask[:, :, sl], t1[:, :, sl])
                # tanh(c)
                th = thpool.tile([128, KC, NB], F32, tag="th")
                nc.scalar.activation(th[:, :, xlo:xhi], cT[:, :, xlo:xhi], AF.Tanh)
                # h update
                for (o, a, si) in segs:
                    sl = slice(o, o + a)
                    t3 = tpool.tile([128, KC, NB], F32, tag="tmp")
                    nc.vector.tensor_mul(t3[:, :, sl], gts[3][:, :, sl], th[:, :, sl])
                    nc.vector.copy_predicated(hT[:, :, sl], mask[:, :, sl], t3[:, :, sl])
                    nc.vector.tensor_copy(hTr[:, :, sl], hT[:, :, sl])

            # ---------------- forward
            w = load_weights("f")
            for t, a1, a2, av in sched["fwd"]:
                lstm_step(w, bias_t["b_uf"][:], bias_t["b_vf"][:], t, a1, a2, av)
            nc.vector.tensor_copy(hsum[:], hT[:])
            nc.gpsimd.memset(hT[:], 0.0)
            nc.gpsimd.memset(cT[:], 0.0)
            nc.vector.tensor_copy(hTr[:], cT[:])
            # ---------------- backward
            w = load_weights("b")
            for t, a1, a2, av in sched["bwd"]:
                lstm_step(w, bias_t["b_ub"][:], bias_t["b_vb"][:], t, a1, a2, av)
            nc.vector.tensor_add(hsum[:], hsum[:], hT[:])

            # ---------------- outputs of LSTM
            nc.sync.dma_start(hu_out[:], hsum[:, :, 0:NU])
            nc.sync.dma_start(hv_out[:], hsum[:, :, NU:NB])

            # ---------------- ranking convnet (fp32)
            ert = convpool.tile([128, KC, n2], F32, tag="ert")
            eat = convpool.tile([128, KC, n2], F32, tag="eat")
            eacct = convpool.tile([128, KC, n2], F32, tag="eacct")
            nc.sync.dma_start(ert[:], er_in[:])
            nc.sync.dma_start(eat[:], ea_in[:])
            nc.sync.dma_start(eacct[:], eacc_in[:])
            k1t = convpool.tile([128, KC, 32], F32, tag="k1t")
            k2t = convpool.tile([128, KC, 2, 32], F32, tag="k2t")
            k3t = convpool.tile([128, KC, 3, 32], F32, tag="k3t")
            cbt = convpool.tile([32, 3], F32, tag="cbt")
            fwt = convpool.tile([32, 3], F32, tag="fwt")
            nc.sync.dma_start(k1t[:], k1_in[:])
            nc.sync.dma_start(k2t[:], k2_in[:])
            nc.sync.dma_start(k3t[:], k3_in[:])
            nc.sync.dma_start(cbt[:], cb_in[:])
            nc.sync.dma_start(fwt[:], fw_in[:])

            def row_ap(r, k):
                if r == 0:
                    return ert[:, k, :]
                if r == 1:
                    return hsum[:, k, n1:NU]
                if r == 2:
                    return eat[:, k, :]
                return eacct[:, k, :]

            def conv_mm(weight_aps_rows, tag):
                # rows: list of (lhsT_ap_per_k(list over kc), row_idx)
                ps = cpspool.tile([32, n2], F32, tag="cps")
                nmm = sum(2 for _ in weight_aps_rows)
                i = 0
                for (w_aps, r) in weight_aps_rows:
                    for k in range(KC):
                        nc.tensor.matmul(ps[:], w_aps[k], row_ap(r, k),
                                         start=(i == 0), stop=(i == nmm - 1))
                        i += 1
                return ps

            def relu_from(ps, cb_idx, tag):
                t = convpool.tile([32, n2], F32, tag=tag, name=f"conv_{tag}")
                nc.scalar.activation(t[:], ps[:], AF.Relu,
                                     bias=cbt[:, cb_idx:cb_idx + 1])
                return t

            c1 = []
            for r in range(4):
                ps = conv_mm([([k1t[:, k, :] for k in range(KC)], r)], "c1")
                c1.append(relu_from(ps, 0, f"c1_{r}"))
            p1l = convpool.tile([32, n2], F32, tag="p1l")
            nc.vector.tensor_max(p1l[:], c1[0][:], c1[1][:])
            nc.vector.tensor_max(p1l[:], p1l[:], c1[2][:])
            p1h = convpool.tile([32, n2], F32, tag="p1h")
            nc.vector.tensor_max(p1h[:], c1[0][:], c1[1][:])
            nc.vector.tensor_max(p1h[:], p1h[:], c1[3][:])

            k2a = [k2t[:, k, 0, :] for k in range(KC)]
            k2b = [k2t[:, k, 1, :] for k in range(KC)]
            w0 = relu_from(conv_mm([(k2a, 0), (k2b, 1)], "w0"), 1, "w0")
            wl1 = relu_from(conv_mm([(k2a, 1), (k2b, 2)], "wl1"), 1, "wl1")
            wh1 = relu_from(conv_mm([(k2a, 1), (k2b, 3)], "wh1"), 1, "wh1")
            p2l = convpool.tile([32, n2], F32, tag="p2l")
            nc.vector.tensor_max(p2l[:], w0[:], wl1[:])
            p2h = convpool.tile([32, n2], F32, tag="p2h")
            nc.vector.tensor_max(p2h[:], w0[:], wh1[:])

            k3a = [k3t[:, k, 0, :] for k in range(KC)]
            k3b = [k3t[:, k, 1, :] for k in range(KC)]
            k3c = [k3t[:, k, 2, :] for k in range(KC)]
            c3l = relu_from(conv_mm([(k3a, 0), (k3b, 1), (k3c, 2)], "c3l"), 2, "c3l")
            c3h = relu_from(conv_mm([(k3a, 0), (k3b, 1), (k3c, 3)], "c3h"), 2, "c3h")

            def score(p1, p2, c3, widx, bias_val, tag):
                ps = cpspool.tile([1, n2], F32, tag="scps")
                nc.tensor.matmul(ps[:], fwt[:, widx[0]:widx[0] + 1], p1[:],
                                 start=True, stop=False)
                nc.tensor.matmul(ps[:], fwt[:, widx[1]:widx[1] + 1], p2[:],
                                 start=False, stop=False)
                nc.tensor.matmul(ps[:], fwt[:, widx[2]:widx[2] + 1], c3[:],
                                 start=False, stop=True)
                t = convpool.tile([1, n2], F32, tag=tag, name=f"score_{tag}")
                nc.scalar.activation(t[:], ps[:], AF.Copy, bias=float(bias_val))
                return t

            low = score(p1l, p2l, c3l, (0, 1, 2), fbias_low, "low")
            high = score(p1h, p2h, c3h, (2, 1, 2), fbias_high, "high")
            nc.sync.dma_start(sc_out[0:1, :], low[:])
            nc.sync.dma_start(sc_out[1:2, :], high[:])

    nc.compile()
    return nc


# ---------------------------------------------------------------- main entry

def kernel(**inputs):
    global LAST_EXEC_NS
    from concourse.bass_utils import run_bass_kernel_spmd

    f32 = np.float32
    q_texts = np.asarray(inputs["q_texts"], dtype=f32)
    q_lens = np.asarray(inputs["q_lens"]).astype(np.int64)
    Q, L, Din = q_texts.shape
    assert Din == D

    idx = {k: np.asarray(inputs[k]).astype(np.int64) for k in
           ["rpos_u", "rpos_v", "rpos_neg", "apos_u", "apos_v", "apos_neg",
            "qpos_u", "qpos_v", "qpos_neg", "rank_r", "rank_a", "rank_acc",
            "rank_q"]}
    B = idx["qpos_u"].shape[0]
    BK = idx["qpos_neg"].shape[0]
    K = BK // B
    BR = idx["rank_q"].shape[0]
    assert B % NCORES == 0 and BR % NCORES == 0

    emb = {k: np.asarray(inputs[k], dtype=f32) for k in
           ["ru_emb", "rv_emb", "au_emb", "av_emb"]}
    wts = {k: np.asarray(inputs[k], dtype=f32) for k in
           ["uWih_f", "uWhh_f", "ub_f", "uWih_b", "uWhh_b", "ub_b",
            "vWih_f", "vWhh_f", "vb_f", "vWih_b", "vWhh_b", "vb_b",
            "k1", "cb1", "k2", "cb2", "k3", "cb3"]}
    fws = {k: float(np.asarray(inputs[k], dtype=f32)) for k in
           ["f1b", "f2b", "f3b"]}
    fwv = {k: np.asarray(inputs[k], dtype=f32) for k in ["f1w", "f2w", "f3w"]}

    # ---------------- dealing / sharding
    u1q, u1l, u1map, a1c, cnt1 = _deal(idx["qpos_u"], q_lens[idx["qpos_u"]], L)
    u2q, u2l, u2map, a2c, cnt2 = _deal(idx["rank_q"], q_lens[idx["rank_q"]], L)

    n1 = B // NCORES
    n2 = BR // NCORES
    NU = n1 + n2
    uniq = np.unique(np.concatenate([idx["qpos_v"], idx["qpos_neg"]]))
    npad = (-len(uniq)) % (2 * NCORES)
    vq_all = np.concatenate([uniq, np.zeros(npad, dtype=np.int64)])
    vl_all = np.concatenate([q_lens[uniq], np.zeros(npad, dtype=np.int64)])
    vq, vl, vmap, avc, cntv = _deal(vq_all, vl_all, L)
    Nv = vq.shape[1]
    NB = NU + Nv

    # per-step schedule (shared across cores)
    fwd, bwd = [], []
    for t in range(L):
        a1, a2, av = int(a1c[t]), int(a2c[t]), int(avc[t])
        if a1 + a2 + av:
            fwd.append((t, a1, a2, av))
    for s in range(L):
        t = L - 1 - s
        a1, a2, av = int(a1c[t]), int(a2c[t]), int(avc[t])
        if a1 + a2 + av:
            bwd.append((t, a1, a2, av))

    fbias_low = fws["f1b"] + fws["f2b"] + fws["f3b"]
    fbias_high = fws["f3b"] + fws["f2b"] + fws["f3b"]
    sched = {"fwd": fwd, "bwd": bwd, "fbias": (fbias_low, fbias_high)}

    key = (L, Nv, n1, n2, tuple(fwd), tuple(bwd), fbias_low, fbias_high)
    nbufs = {"x": 2, "g": 2, "tmp": 3, "psu": 3, "psv": 3}
    if key not in _PROG_CACHE:
        _PROG_CACHE[key] = _build_program(L, Nv, n1, n2, sched, nbufs)
    nc = _PROG_CACHE[key]

    # ---------------- per-core inputs
    wpacked = {
        "wih_uf": _pack_wT(wts["uWih_f"]), "whh_uf": _pack_wT(wts["uWhh_f"]),
        "wih_ub": _pack_wT(wts["uWih_b"]), "whh_ub": _pack_wT(wts["uWhh_b"]),
        "wih_vf": _pack_wT(wts["vWih_f"]), "whh_vf": _pack_wT(wts["vWhh_f"]),
        "wih_vb": _pack_wT(wts["vWih_b"]), "whh_vb": _pack_wT(wts["vWhh_b"]),
        "b_uf": np.ascontiguousarray(wts["ub_f"].reshape(8, 128).T),
        "b_ub": np.ascontiguousarray(wts["ub_b"].reshape(8, 128).T),
        "b_vf": np.ascontiguousarray(wts["vb_f"].reshape(8, 128).T),
        "b_vb": np.ascontiguousarray(wts["vb_b"].reshape(8, 128).T),
    }
    iota = np.broadcast_to(np.arange(NB, dtype=f32), (128, KC, NB))
    iota = np.ascontiguousarray(iota)
    k1t = _pack_rows_T(wts["k1"])  # [128,2,32]
    k2t = np.stack([_pack_rows_T(wts["k2"][:, j, :]) for j in range(2)], axis=2)
    k3t = np.stack([_pack_rows_T(wts["k3"][:, j, :]) for j in range(3)], axis=2)
    cbt = np.stack([wts["cb1"], wts["cb2"], wts["cb3"]], axis=1)
    fwt = np.stack([fwv["f1w"], fwv["f2w"], fwv["f3w"]], axis=1)

    er_rows = emb["ru_emb"][idx["rank_r"]]
    ea_rows = emb["au_emb"][idx["rank_a"]]
    eacc_rows = emb["au_emb"][idx["rank_acc"]]

    in_maps = []
    for c in range(NCORES):
        qcols = np.concatenate([u1q[c], u2q[c], vq[c]])
        g = q_texts[qcols]                      # [NB, L, 256]
        x_core = np.ascontiguousarray(
            g.transpose(1, 2, 0).reshape(L, KC, 128, NB).transpose(0, 2, 1, 3))
        cnts = np.zeros((128, L, 3), dtype=f32)
        cnts[:, :, 0] = cnt1[c][None, :]
        cnts[:, :, 1] = n1 + cnt2[c][None, :]
        cnts[:, :, 2] = NU + cntv[c][None, :]
        # per-core permuted rank embeddings: col j on core c is rank item
        # with (u2map cores==c, cols==j)
        sel = np.where(u2map[0] == c)[0]
        ordr = sel[np.argsort(u2map[1][sel])]
        m = {"x_in": x_core, "iota": iota, "cnts": cnts,
             "er": _pack_rows_T(er_rows[ordr]),
             "ea": _pack_rows_T(ea_rows[ordr]),
             "eacc": _pack_rows_T(eacc_rows[ordr]),
             "k1t": k1t, "k2t": np.ascontiguousarray(k2t),
             "k3t": np.ascontiguousarray(k3t),
             "cbt": np.ascontiguousarray(cbt),
             "fwt": np.ascontiguousarray(fwt)}
        m.update(wpacked)
        in_maps.append(m)

    trace = os.environ.get("NERANK_TRACE", "") == "1"
    if trace:
        _install_ntff_stub()
    res = run_bass_kernel_spmd(nc, in_maps, list(range(NCORES)), trace=trace)
    if trace:
        LAST_EXEC_NS = res.exec_time_ns

    # ---------------- host assembly (float64)
    hu = [_unpack_hT(res.results[c]["hu_out"]) for c in range(NCORES)]
    hv = [_unpack_hT(res.results[c]["hv_out"]) for c in range(NCORES)]
    scores = [res.results[c]["sc_out"] for c in range(NCORES)]

    embed_qu = np.empty((B, 256), dtype=f32)
    cu, co = u1map
    for b in range(B):
        embed_qu[b] = hu[cu[b]][co[b]]

    hq_table = np.empty((len(uniq), 256), dtype=f32)
    cv, cov = vmap
    for i in range(len(uniq)):
        hq_table[i] = hv[cv[i]][cov[i]]
    lut = np.searchsorted(uniq, idx["qpos_v"])
    embed_qv = hq_table[lut]
    lutn = np.searchsorted(uniq, idx["qpos_neg"])
    neg_qv = hq_table[lutn]

    embed_u = (emb["ru_emb"][idx["rpos_u"]] + emb["au_emb"][idx["apos_u"]]
               + embed_qu).astype(np.float64)
    embed_v = (emb["rv_emb"][idx["rpos_v"]] + emb["av_emb"][idx["apos_v"]]
               + embed_qv).astype(np.float64)
    neg_v = (emb["rv_emb"][idx["rpos_neg"]] + emb["av_emb"][idx["apos_neg"]]
             + neg_qv).astype(np.float64).reshape(B, K, 256)

    S1 = float(np.sum(embed_u * embed_v))
    S2 = float(np.einsum("bkd,bd->", neg_v, embed_u))
    ne_loss = float(_logsig(S1) + _logsig(-S2))

    rank_loss = np.empty(BR, dtype=np.float64)
    cr, cro = u2map
    for r in range(BR):
        lo = scores[cr[r]][0, cro[r]]
        hi = scores[cr[r]][1, cro[r]]
        rank_loss[r] = _logsig(float(lo) - float(hi))

    return (ne_loss + rank_loss).astype(f32)


def _install_ntff_stub():
    import sys, types
    try:
        from antenv import axon_hooks  # noqa: F401
        return
    except ImportError:
        pass
    if "/root/.axon_site" not in sys.path:
        sys.path.insert(0, "/root/.axon_site")
    from trn_agent_boot.trn_boot import _ntff_profile_via_ctypes
    hook = _ntff_profile_via_ctypes("/opt/axon/libaxon_pjrt.so")
    mod = types.ModuleType("antenv.axon_hooks")
    _state = {"hook": hook}
    mod.set_axon_ntff_profile_hook = lambda h: _state.__setitem__("hook", h)
    mod.get_axon_ntff_profile_hook = lambda: _state["hook"]
    sys.modules["antenv.axon_hooks"] = mod
    import antenv
    antenv.axon_hooks = mod
